# revision 8
# baseline (speedup 1.0000x reference)
"""Trainium2 Bass kernel for a pre-LN transformer block.

Block: y = x + FFN(LN2(x + Attn(LN1(x))))  with causal 8-head attention.
Shapes: x [64, 256, 512], 8 heads x 64 dim, FFN 512->2048->512, fp32 I/O.

Sharding: data-parallel over batch, 8 sequences per NeuronCore, no
collectives.  Each core runs the identical program on its batch shard.

On-chip dataflow (per batch of one core):
  - activations that feed matmuls are kept TRANSPOSED [channel, token] so
    DRAM weight matrices [c, d] serve directly as matmul lhsT
  - LN / softmax run in [token, channel] layout (free-dim reductions)
  - matmul operands bf16, accumulation fp32 in PSUM
  - LN gamma folded into weights on host; beta handled via exact bias terms
  - rsqrt computed as exp(-0.5*ln(v)) so ACT stays in one table set
"""

import numpy as np

import concourse.bacc as bacc
import concourse.bass as bass
import concourse.mybir as mybir
import concourse.tile as tile
from concourse.bass_utils import run_bass_kernel_spmd

F32 = mybir.dt.float32
BF16 = mybir.dt.bfloat16
NP_BF16 = mybir.dt.np(BF16)

B, T, C = 64, 256, 512
H, D = 8, 64
FF = 4 * C
NCORES = 8
NB = B // NCORES  # batches per core
EPS = 1e-5
SCALE = float(C) ** -0.5
AF = mybir.ActivationFunctionType
ALU = mybir.AluOpType


def _ln_tc(nc, wp, x_tile, h_out, eps_s):
    """LayerNorm core (x - mean) * rsqrt(var + eps) for one [128, C] tile.
    Writes bf16 h_out.  gamma/beta are folded into the weights elsewhere."""
    st6 = wp.tile([128, 6], F32, tag="ln_st6", bufs=2)
    mv = wp.tile([128, 2], F32, tag="ln_mv", bufs=2)
    lnv = wp.tile([128, 1], F32, tag="ln_lnv", bufs=2)
    rstd = wp.tile([128, 1], F32, tag="ln_rstd", bufs=2)
    nc.vector.bn_stats(st6[:], x_tile)
    nc.vector.bn_aggr(mv[:], st6[:])
    # rstd = exp(-0.5 * ln(var + eps)); Ln and Exp share one ACT table set
    nc.scalar.activation(lnv[:], mv[:, 1:2], AF.Ln, bias=eps_s[:])
    nc.scalar.activation(rstd[:], lnv[:], AF.Exp, scale=-0.5)
    # h = (x - mean) * rstd  in one dual-op tensor_scalar
    nc.vector.tensor_scalar(
        h_out, x_tile, mv[:, 0:1], rstd[:], ALU.subtract, ALU.mult
    )


def build_nc():
    nc = bacc.Bacc(
        "TRN2",
        target_bir_lowering=False,
        debug=False,
        num_devices=NCORES,
    )

    x_d = nc.dram_tensor("x_s", [NB, T, C], F32, kind="ExternalInput")
    wq_d = nc.dram_tensor("wq", [C, C], BF16, kind="ExternalInput")
    wk_d = nc.dram_tensor("wk", [C, C], BF16, kind="ExternalInput")
    wv_d = nc.dram_tensor("wv", [C, C], BF16, kind="ExternalInput")
    pw_d = nc.dram_tensor("pw", [C, C], BF16, kind="ExternalInput")
    w1_d = nc.dram_tensor("w1", [C, FF], BF16, kind="ExternalInput")
    w2_d = nc.dram_tensor("w2", [FF, C], BF16, kind="ExternalInput")
    bq_d = nc.dram_tensor("bq_t", [128, 4], F32, kind="ExternalInput")
    bk_d = nc.dram_tensor("bk_t", [128, 4], F32, kind="ExternalInput")
    bvb_d = nc.dram_tensor("bv_bc", [128, C], F32, kind="ExternalInput")
    pb_d = nc.dram_tensor("pb_t", [128, 4], F32, kind="ExternalInput")
    b1_d = nc.dram_tensor("b1_t", [128, 16], F32, kind="ExternalInput")
    b2_d = nc.dram_tensor("b2_t", [128, 4], F32, kind="ExternalInput")
    msk_d = nc.dram_tensor("mask128", [128, 128], F32, kind="ExternalInput")
    idb_d = nc.dram_tensor("id_bf", [128, 128], BF16, kind="ExternalInput")
    idf_d = nc.dram_tensor("id_f32", [128, 128], F32, kind="ExternalInput")
    y_d = nc.dram_tensor("y_s", [NB, T, C], F32, kind="ExternalOutput")

    with tile.TileContext(nc) as tc:
        with (
            tc.tile_pool(name="const", bufs=1) as cp,
            tc.tile_pool(name="work", bufs=2) as wp,
            tc.tile_pool(name="psum", bufs=2, space="PSUM") as pp,
        ):
            # ---- persistent constants -------------------------------------
            wq_s = cp.tile([128, 4, C], BF16)  # (c_loc, cb, d_cat)
            wk_s = cp.tile([128, 4, C], BF16)
            wv_s = cp.tile([128, 4, C], BF16)
            pw_s = cp.tile([128, 4, C], BF16)  # (c_loc, cb, e)
            w1_s = cp.tile([128, 4, FF], BF16)  # (c_loc, cb, f)
            w2_s = cp.tile([128, 16, C], BF16)  # (f_loc, fb, e)
            bq_s = cp.tile([128, 4], F32)
            bk_s = cp.tile([128, 4], F32)
            bvb_s = cp.tile([128, C], F32)
            pb_s = cp.tile([128, 4], F32)
            b1_s = cp.tile([128, 16], F32)
            b2_s = cp.tile([128, 4], F32)
            msk_s = cp.tile([128, 128], F32)
            idb_s = cp.tile([128, 128], BF16)
            idf_s = cp.tile([128, 128], F32)
            eps_s = cp.tile([128, 1], F32)

            nc.sync.dma_start(wq_s[:], wq_d.ap().rearrange("(cb c) d -> c cb d", c=128))
            nc.sync.dma_start(wk_s[:], wk_d.ap().rearrange("(cb c) d -> c cb d", c=128))
            nc.sync.dma_start(wv_s[:], wv_d.ap().rearrange("(cb c) d -> c cb d", c=128))
            nc.sync.dma_start(pw_s[:], pw_d.ap().rearrange("(cb c) d -> c cb d", c=128))
            nc.sync.dma_start(w1_s[:], w1_d.ap().rearrange("(cb c) f -> c cb f", c=128))
            nc.sync.dma_start(w2_s[:], w2_d.ap().rearrange("(fb f) e -> f fb e", f=128))
            nc.sync.dma_start(bq_s[:], bq_d.ap())
            nc.sync.dma_start(bk_s[:], bk_d.ap())
            nc.sync.dma_start(bvb_s[:], bvb_d.ap())
            nc.sync.dma_start(pb_s[:], pb_d.ap())
            nc.sync.dma_start(b1_s[:], b1_d.ap())
            nc.sync.dma_start(b2_s[:], b2_d.ap())
            nc.sync.dma_start(msk_s[:], msk_d.ap())
            nc.sync.dma_start(idb_s[:], idb_d.ap())
            nc.sync.dma_start(idf_s[:], idf_d.ap())
            nc.gpsimd.memset(eps_s[:], EPS)

            for nb in range(NB):
                _emit_batch(
                    nc, wp, pp, nb, x_d, y_d,
                    wq_s, wk_s, wv_s, pw_s, w1_s, w2_s,
                    bq_s, bk_s, bvb_s, pb_s, b1_s, b2_s,
                    msk_s, idb_s, idf_s, eps_s,
                )

    nc.compile()
    return nc


def _emit_batch(
    nc, wp, pp, nb, x_d, y_d,
    wq_s, wk_s, wv_s, pw_s, w1_s, w2_s,
    bq_s, bk_s, bvb_s, pb_s, b1_s, b2_s,
    msk_s, idb_s, idf_s, eps_s,
):
    # ---- load x, LN1 ----------------------------------------------------
    xa = []
    h = []
    for tcb in range(2):
        xt = wp.tile([128, C], F32, tag=f"xa{tcb}", bufs=2)
        nc.sync.dma_start(xt[:], x_d[nb, tcb * 128:(tcb + 1) * 128, :])
        ht = wp.tile([128, C], BF16, tag=f"h{tcb}", bufs=2)
        _ln_tc(nc, wp, xt[:], ht[:], eps_s)
        xa.append(xt)
        h.append(ht)

    # ---- hT via PE transpose: (c_loc, cb, t) ----------------------------
    hT = wp.tile([128, 4, T], BF16, tag="hT", bufs=2)
    for cb in range(4):
        for tcb in range(2):
            pt = pp.tile([128, 128], BF16, tag="ptr", bufs=2)
            nc.tensor.transpose(
                pt[:], h[tcb][:, cb * 128:(cb + 1) * 128], idb_s[:]
            )
            nc.vector.tensor_copy(hT[:, cb, tcb * 128:(tcb + 1) * 128], pt[:])

    # ---- QKV projections ------------------------------------------------
    # qT/kT: (d_loc, db, t) = W.T @ hT ; v: (s_loc, sc, d_cat) = h @ Wv
    qT = wp.tile([128, 4, T], BF16, tag="qT", bufs=2)
    kT = wp.tile([128, 4, T], BF16, tag="kT", bufs=2)
    for w_s, b_s, dst in ((wq_s, bq_s, qT), (wk_s, bk_s, kT)):
        for db in range(4):
            ps = pp.tile([128, T], F32, tag="pmm", bufs=3)
            for cb in range(4):
                nc.tensor.matmul(
                    ps[:],
                    w_s[:, cb, db * 128:(db + 1) * 128],
                    hT[:, cb, :],
                    start=(cb == 0),
                    stop=(cb == 3),
                )
            nc.scalar.add(dst[:, db, :], ps[:], b_s[:, db:db + 1])

    v = wp.tile([128, 2, C], BF16, tag="v", bufs=2)
    for sc in range(2):
        ps = pp.tile([128, C], F32, tag="pmm", bufs=3)
        for cb in range(4):
            nc.tensor.matmul(
                ps[:],
                hT[:, cb, sc * 128:(sc + 1) * 128],
                wv_s[:, cb, :],
                start=(cb == 0),
                stop=(cb == 3),
            )
        nc.vector.tensor_tensor(v[:, sc, :], ps[:], bvb_s[:], ALU.add)

    # ---- attention per head --------------------------------------------
    attT = wp.tile([128, 4, T], BF16, tag="attT", bufs=2)
    for hh in range(8):
        po = (hh % 2) * 64  # partition offset of this head's d-rows
        db = hh // 2
        kh = kT[po:po + 64, db, :]
        qh = qT[po:po + 64, db, :]

        # scores (kq^T, scaled inside exp), causal-skipped
        ps0 = pp.tile([128, 128], F32, tag="psc", bufs=3)
        nc.tensor.matmul(ps0[:], kh[:, 0:128], qh[:, 0:128], start=True, stop=True)
        ps1 = pp.tile([128, T], F32, tag="psc", bufs=3)
        nc.tensor.matmul(ps1[:], kh[:, 128:256], qh[:, :], start=True, stop=True)

        # softmax: exp -> mask -> row-sum -> reciprocal -> scale
        wei0 = wp.tile([128, 128], BF16, tag="wei0", bufs=3)
        wei1 = wp.tile([128, T], BF16, tag="wei1", bufs=3)
        rs0 = wp.tile([128, 1], F32, tag="rs0", bufs=3)
        rsA = wp.tile([128, 1], F32, tag="rsA", bufs=3)
        rsB = wp.tile([128, 1], F32, tag="rsB", bufs=3)
        rs1 = wp.tile([128, 1], F32, tag="rs1", bufs=3)
        ex0 = wp.tile([128, 128], F32, tag="ex0", bufs=3)
        ex1 = wp.tile([128, 128], F32, tag="ex1", bufs=3)

        nc.scalar.activation(ex0[:], ps0[:], AF.Exp, scale=SCALE)
        nc.vector.tensor_tensor(wei0[:], ex0[:], msk_s[:], ALU.mult)
        nc.vector.tensor_reduce(rs0[:], wei0[:], mybir.AxisListType.X, ALU.add)
        nc.scalar.activation(
            wei1[:, 0:128], ps1[:, 0:128], AF.Exp, scale=SCALE, accum_out=rsA[:]
        )
        nc.scalar.activation(ex1[:], ps1[:, 128:256], AF.Exp, scale=SCALE)
        nc.vector.tensor_tensor(wei1[:, 128:256], ex1[:], msk_s[:], ALU.mult)
        nc.vector.tensor_reduce(
            rsB[:], wei1[:, 128:256], mybir.AxisListType.X, ALU.add
        )
        nc.vector.tensor_tensor(rs1[:], rsA[:], rsB[:], ALU.add)
        r0 = wp.tile([128, 1], F32, tag="r0", bufs=3)
        r1 = wp.tile([128, 1], F32, tag="r1", bufs=3)
        nc.vector.reciprocal(r0[:], rs0[:])
        nc.vector.reciprocal(r1[:], rs1[:])
        nc.gpsimd.tensor_scalar_mul(wei0[:], wei0[:], r0[:])
        nc.gpsimd.tensor_scalar_mul(wei1[:], wei1[:], r1[:])

        # transpose wei -> weiT (3 causal blocks)
        weiT0 = wp.tile([128, T], BF16, tag="weiT0", bufs=3)  # s0, all t
        weiT1 = wp.tile([128, 128], BF16, tag="weiT1", bufs=3)  # s1, t1
        for src, dst in (
            (wei0[:], weiT0[:, 0:128]),
            (wei1[:, 0:128], weiT0[:, 128:256]),
            (wei1[:, 128:256], weiT1[:]),
        ):
            pt = pp.tile([128, 128], BF16, tag="ptr", bufs=2)
            nc.tensor.transpose(pt[:], src, idb_s[:])
            nc.vector.tensor_copy(dst, pt[:])

        # attT_h: lhsT=v[s,d_h], rhs=weiT[s,t]
        pa = pp.tile([64, T], F32, tag="psc", bufs=3)
        nc.tensor.matmul(
            pa[:, :], v[:, 0, hh * 64:(hh + 1) * 64], weiT0[:],
            start=True, stop=False, skip_group_check=True,
        )
        nc.tensor.matmul(
            pa[:, 128:256], v[:, 1, hh * 64:(hh + 1) * 64], weiT1[:],
            start=False, stop=True, skip_group_check=True,
        )
        nc.scalar.copy(attT[po:po + 64, db, :], pa[:])

    # ---- proj + residual-1 ---------------------------------------------
    y1T = wp.tile([128, 4, T], F32, tag="y1T", bufs=2)
    for eb in range(4):
        ps = pp.tile([128, T], F32, tag="pmm", bufs=3)
        for cb in range(4):
            nc.tensor.matmul(
                ps[:],
                pw_s[:, cb, eb * 128:(eb + 1) * 128],
                attT[:, cb, :],
                start=(cb == 0),
                stop=(cb == 3),
            )
        nc.scalar.add(y1T[:, eb, :], ps[:], pb_s[:, eb:eb + 1])

    y1 = []
    for tcb in range(2):
        y1t = wp.tile([128, C], F32, tag=f"y1_{tcb}", bufs=2)
        for cb in range(4):
            pt = pp.tile([128, 128], F32, tag="ptr", bufs=2)
            nc.tensor.transpose(
                pt[:], y1T[:, cb, tcb * 128:(tcb + 1) * 128], idf_s[:]
            )
            nc.vector.tensor_tensor(
                y1t[:, cb * 128:(cb + 1) * 128],
                xa[tcb][:, cb * 128:(cb + 1) * 128],
                pt[:],
                ALU.add,
            )
        y1.append(y1t)

    # ---- LN2 + FFN + residual-2 ----------------------------------------
    h2 = []
    for tcb in range(2):
        h2t = wp.tile([128, C], BF16, tag=f"h2_{tcb}", bufs=2)
        _ln_tc(nc, wp, y1[tcb][:], h2t[:], eps_s)
        h2.append(h2t)

    h2T = wp.tile([128, 4, T], BF16, tag="h2T", bufs=2)
    for cb in range(4):
        for tcb in range(2):
            pt = pp.tile([128, 128], BF16, tag="ptr", bufs=2)
            nc.tensor.transpose(
                pt[:], h2[tcb][:, cb * 128:(cb + 1) * 128], idb_s[:]
            )
            nc.vector.tensor_copy(h2T[:, cb, tcb * 128:(tcb + 1) * 128], pt[:])

    zT = wp.tile([128, 16, T], BF16, tag="zT", bufs=2)
    for fb in range(16):
        ps = pp.tile([128, T], F32, tag="pmm", bufs=3)
        for cb in range(4):
            nc.tensor.matmul(
                ps[:],
                w1_s[:, cb, fb * 128:(fb + 1) * 128],
                h2T[:, cb, :],
                start=(cb == 0),
                stop=(cb == 3),
            )
        nc.scalar.activation(zT[:, fb, :], ps[:], AF.Relu, bias=b1_s[:, fb:fb + 1])

    yT = wp.tile([128, 4, T], F32, tag="yT", bufs=2)
    for eb in range(4):
        ps = pp.tile([128, T], F32, tag="pmm", bufs=3)
        for fb in range(16):
            nc.tensor.matmul(
                ps[:],
                w2_s[:, fb, eb * 128:(eb + 1) * 128],
                zT[:, fb, :],
                start=(fb == 0),
                stop=(fb == 15),
            )
        nc.scalar.add(yT[:, eb, :], ps[:], b2_s[:, eb:eb + 1])

    for tcb in range(2):
        ot = wp.tile([128, C], F32, tag=f"out{tcb}", bufs=2)
        for cb in range(4):
            pt = pp.tile([128, 128], F32, tag="ptr", bufs=2)
            nc.tensor.transpose(
                pt[:], yT[:, cb, tcb * 128:(tcb + 1) * 128], idf_s[:]
            )
            nc.vector.tensor_tensor(
                ot[:, cb * 128:(cb + 1) * 128],
                y1[tcb][:, cb * 128:(cb + 1) * 128],
                pt[:],
                ALU.add,
            )
        nc.sync.dma_start(y_d[nb, tcb * 128:(tcb + 1) * 128, :], ot[:])


_NC_CACHE = {}


def _get_nc():
    if "nc" not in _NC_CACHE:
        _NC_CACHE["nc"] = build_nc()
    return _NC_CACHE["nc"]


def _prep_inputs(x, Wk, Wq, Wv, proj_w, proj_b, ln1_g, ln1_b, W1, b1, W2, b2,
                 ln2_g, ln2_b):
    """Host-side prep: fold LN gamma into weights, compute exact beta bias
    terms, concat heads, cast matmul operands to bf16."""
    f32 = np.float32
    g1 = np.asarray(ln1_g, f32)
    be1 = np.asarray(ln1_b, f32)
    g2 = np.asarray(ln2_g, f32)
    be2 = np.asarray(ln2_b, f32)

    def cat_heads(w):  # [H, C, D] -> [C, H*D]
        return np.ascontiguousarray(
            np.asarray(w, f32).transpose(1, 0, 2).reshape(C, C)
        )

    wq_c = cat_heads(Wq)
    wk_c = cat_heads(Wk)
    wv_c = cat_heads(Wv)
    bias_q = be1 @ wq_c  # [C]
    bias_k = be1 @ wk_c
    bias_v = be1 @ wv_c
    wq_eff = (g1[:, None] * wq_c).astype(NP_BF16)
    wk_eff = (g1[:, None] * wk_c).astype(NP_BF16)
    wv_eff = (g1[:, None] * wv_c).astype(NP_BF16)

    w1f = np.asarray(W1, f32)
    b1_eff = np.asarray(b1, f32) + be2 @ w1f
    w1_eff = (g2[:, None] * w1f).astype(NP_BF16)

    common = dict(
        wq=wq_eff,
        wk=wk_eff,
        wv=wv_eff,
        pw=np.asarray(proj_w, f32).astype(NP_BF16),
        w1=w1_eff,
        w2=np.asarray(W2, f32).astype(NP_BF16),
        bq_t=np.ascontiguousarray(bias_q.reshape(4, 128).T.astype(f32)),
        bk_t=np.ascontiguousarray(bias_k.reshape(4, 128).T.astype(f32)),
        bv_bc=np.ascontiguousarray(np.tile(bias_v.astype(f32), (128, 1))),
        pb_t=np.ascontiguousarray(
            np.asarray(proj_b, f32).reshape(4, 128).T.astype(f32)
        ),
        b1_t=np.ascontiguousarray(b1_eff.reshape(16, 128).T.astype(f32)),
        b2_t=np.ascontiguousarray(
            np.asarray(b2, f32).reshape(4, 128).T.astype(f32)
        ),
        mask128=np.tril(np.ones((128, 128), f32)),
        id_bf=np.eye(128, dtype=NP_BF16),
        id_f32=np.eye(128, dtype=f32),
    )
    return np.asarray(x, f32), common


def kernel(**inputs) -> np.ndarray:
    x_full, common = _prep_inputs(**inputs)
    nc = _get_nc()
    in_maps = []
    for core in range(NCORES):
        m = dict(common)
        m["x_s"] = np.ascontiguousarray(x_full[core * NB:(core + 1) * NB])
        in_maps.append(m)
    res = run_bass_kernel_spmd(nc, in_maps, list(range(NCORES)))
    return np.concatenate([r["y_s"] for r in res.results], axis=0)


if __name__ == "__main__":
    import reference

    inputs = {k: np.asarray(v) for k, v in reference.setup_inputs().items()}
    out = kernel(**inputs)
    exp = np.asarray(reference.reference(**inputs))
    err = np.abs(out - exp).max() / (np.abs(exp).max() + 1e-9)
    print("max-rel err:", err)
